# revision 1
# baseline (speedup 1.0000x reference)
"""Multi-head attention (bs=2, seq=2048, d_model=1024, 16 heads) on 8 NeuronCores.

Sharding: core = b*4 + g  (b = batch 0..1, g = head-group 0..3, 4 heads each).
Per core, for batch b and head slice s256 = [256g, 256g+256):
  qhT [256, 2048] = (0.125*W_q[s256]) @ q[b].T      (scores scale folded into W_q)
  khT [256, 2048] = W_k[s256] @ k[b].T
  vh  [2048, 260] = v[b] @ W_v[s256].T              (+ ones column per head)
  per head: S^T = khT-slice.T @ qhT -> exp -> P^T (bf16)
            attnU^T[65, sq] = vh_aug.T @ P^T        (row 64 = softmax sums)
            normalize with PE-transposed reciprocal sums
  out_partial [2048, 1024] = attnN @ W_o[:, s256].T   (f32)
Host sums the 4 partials per batch and adds b_o.
Head pairs (2t, 2t+1) interleave their K=64 S^T matmuls on PE row groups
0-1 / 2-3 so the systolic array runs both concurrently.
"""

import sys

sys.path.insert(0, "/opt/trn_rl_repo")

import numpy as np
import ml_dtypes

import concourse.bass as bass
import concourse.mybir as mybir
import concourse.tile as tile
from concourse import bacc
from concourse.bass_utils import run_bass_kernel_spmd
from concourse.masks import make_identity

BF16 = ml_dtypes.bfloat16
F32 = mybir.dt.float32
BF = mybir.dt.bfloat16

SEQ = 2048
DM = 1024
DSL = 256            # head dims per core
NT = SEQ // 128      # 16 seq tiles
NC4 = 4              # seq chunks of 512

_cache = {}


def _build(reps=1):
    nc = bacc.Bacc(None, target_bir_lowering=False, debug=False)
    with tile.TileContext(nc) as tc:
        with tc.tile_pool(name="dram", bufs=1, space="DRAM") as dram:
            qT_d = dram.tile([128, 8, SEQ], BF, kind="ExternalInput", tag="qT")
            kT_d = dram.tile([128, 8, SEQ], BF, kind="ExternalInput", tag="kT")
            vT_d = dram.tile([128, 8, SEQ], BF, kind="ExternalInput", tag="vT")
            wq_d = dram.tile([128, 8, DSL], BF, kind="ExternalInput", tag="wq")
            wk_d = dram.tile([128, 8, DSL], BF, kind="ExternalInput", tag="wk")
            wv_d = dram.tile([128, 8, DSL], BF, kind="ExternalInput", tag="wv")
            wo_d = dram.tile([128, 2, DM], BF, kind="ExternalInput", tag="wo")
            out_d = dram.tile([SEQ, DM], F32, kind="ExternalOutput", tag="out")

            with tc.tile_pool(name="const", bufs=1) as cp:
                wo_sb = cp.tile([128, 2, DM], BF, tag="cwo")
                ident = cp.tile([128, 128], F32, tag="cid")
                ones = cp.tile([1, 64], F32, tag="cones")
                nc.scalar.dma_start(wo_sb[:], wo_d[:])
                make_identity(nc, ident[:])
                nc.gpsimd.memset(ones[:], 1.0)

                with tc.tile_pool(name="persist", bufs=1) as pp:
                    qh_sb = pp.tile([128, 2, SEQ], BF, tag="qh")
                    kh_sb = pp.tile([128, 2, SEQ], BF, tag="kh")
                    vh_sb = pp.tile([128, NT, 260], BF, tag="vh")
                    vh_ones = vh_sb[:].rearrange(
                        "p m (h x) -> p m h x", h=4
                    )[:, :, :, 64:65]
                    nc.vector.memset(vh_ones, 1.0)

                    for _rep in range(reps):
                        with (
                            tc.tile_pool(name="aps", bufs=1, space="PSUM") as aps,
                            tc.tile_pool(name="ptp", bufs=1) as ptp,
                        ):
                            # ---------------- q/k projections ----------------
                            with tc.tile_pool(name="ioqk", bufs=1) as io:
                                wq_sb = io.tile([128, 8, DSL], BF, tag="cwq")
                                wk_sb = io.tile([128, 8, DSL], BF, tag="cwk")
                                nc.scalar.dma_start(wq_sb[:], wq_d[:])
                                nc.scalar.dma_start(wk_sb[:], wk_d[:])
                                qt_sb = io.tile([128, 8, SEQ], BF, tag="qt")
                                kt_sb = io.tile([128, 8, SEQ], BF, tag="kt")
                                nc.sync.dma_start(qt_sb[:], qT_d[:])
                                nc.sync.dma_start(kt_sb[:], kT_d[:])
                                for m in range(2):
                                    for (w_sb, x_sb, o_sb) in ((wq_sb, qt_sb, qh_sb), (wk_sb, kt_sb, kh_sb)):
                                        for n in range(NC4):
                                            ps = aps.tile([128, 512], F32, tag="av", bufs=4, name=f"pj{m}{n}")
                                            for j in range(8):
                                                nc.tensor.matmul(
                                                    ps[:],
                                                    w_sb[:, j, m * 128 : (m + 1) * 128],
                                                    x_sb[:, j, n * 512 : (n + 1) * 512],
                                                    start=(j == 0),
                                                    stop=(j == 7),
                                                )
                                            nc.vector.tensor_copy(
                                                o_sb[:, m, n * 512 : (n + 1) * 512], ps[:]
                                            )

                            with (
                                tc.tile_pool(name="iov", bufs=1) as iov,
                                tc.tile_pool(name="asb", bufs=1) as ap,
                            ):
                                att_sb = ap.tile([128, 2, SEQ], BF, tag="att")
                                wv_sb = iov.tile([128, 8, DSL], BF, tag="cwv")
                                nc.scalar.dma_start(wv_sb[:], wv_d[:])
                                vt_sb = iov.tile([128, 8, SEQ], BF, tag="vt")
                                nc.gpsimd.dma_start(vt_sb[:], vT_d[:])

                                pts = [[] for _ in range(4)]
                                avs = {}
                                u_saved = {}

                                def s_step(h, m):
                                    t, p0 = h // 2, 64 * (h % 2)
                                    pt = ptp.tile([128, SEQ], BF, tag="pt", bufs=18,
                                                  name=f"pt{h}_{m}")
                                    for c in range(2):
                                        s_ps = aps.tile([128, 1024], F32, tag="s", bufs=2,
                                                        name=f"s{h}_{m}{c}")
                                        for n in range(2):
                                            nn = 2 * c + n
                                            nc.tensor.matmul(
                                                s_ps[:, n * 512 : (n + 1) * 512],
                                                kh_sb[p0 : p0 + 64, t, m * 128 : (m + 1) * 128],
                                                qh_sb[p0 : p0 + 64, t, nn * 512 : (nn + 1) * 512],
                                                start=True,
                                                stop=True,
                                            )
                                        nc.scalar.activation(
                                            pt[:, c * 1024 : (c + 1) * 1024],
                                            s_ps[:],
                                            mybir.ActivationFunctionType.Exp,
                                        )
                                    pts[h].append(pt)

                                def av_step(h, m):
                                    for n in range(NC4):
                                        nc.tensor.matmul(
                                            avs[h][n][0:65, :],
                                            vh_sb[:, m, 65 * h : 65 * h + 65],
                                            pts[h][m][:, n * 512 : (n + 1) * 512],
                                            start=(m == 0),
                                            stop=(m == NT - 1),
                                        )

                                def ucopy(h):
                                    u_sb = ap.tile([64, SEQ], BF, tag="u", bufs=3, name=f"u{h}")
                                    scs = []
                                    for n in range(NC4):
                                        nc.vector.tensor_copy(
                                            u_sb[:, n * 512 : (n + 1) * 512], avs[h][n][0:64, :]
                                        )
                                        sc = ap.tile([65, 512], F32, tag="sc", bufs=6, name=f"sc{h}{n}")
                                        nc.vector.tensor_copy(sc[64:65, :], avs[h][n][64:65, :])
                                        scs.append(sc)
                                    u_saved[h] = (u_sb, scs)

                                def normrest(h):
                                    t, hh = h // 2, h % 2
                                    u_sb, scs = u_saved[h]
                                    sT = aps.tile([128, NT], F32, tag="s", bufs=2, name=f"sT{h}")
                                    for m in range(NT):
                                        nc.tensor.transpose(
                                            sT[:, m : m + 1],
                                            scs[m // 4][64:65, (m % 4) * 128 : (m % 4 + 1) * 128],
                                            ident[64:65, 64:65],
                                        )
                                    rT = ap.tile([128, NT], F32, tag="rT", bufs=2, name=f"rT{h}")
                                    nc.vector.reciprocal(rT[:], sT[:])
                                    stage = (
                                        ap.tile([64, SEQ], BF, tag="u", bufs=3, name=f"stg{h}")
                                        if hh
                                        else None
                                    )
                                    for n in range(NC4):
                                        row = aps.tile([1, 512], F32, tag="s", bufs=2, name=f"row{h}{n}")
                                        for i in range(4):
                                            nc.tensor.transpose(
                                                row[0:1, i * 128 : (i + 1) * 128],
                                                rT[:, 4 * n + i : 4 * n + i + 1],
                                                ident[:, 0:128],
                                            )
                                        rs = ap.tile([1, 512], F32, tag="rs", bufs=2, name=f"rs{h}{n}")
                                        nc.vector.tensor_copy(rs[:], row[:])
                                        bc = aps.tile([64, 512], F32, tag="s", bufs=2, name=f"bc{h}{n}")
                                        nc.tensor.matmul(
                                            bc[:], ones[0:1, :], rs[0:1, :],
                                            start=True, stop=True,
                                        )
                                        tgt = (
                                            stage[:, n * 512 : (n + 1) * 512]
                                            if hh
                                            else att_sb[0:64, t, n * 512 : (n + 1) * 512]
                                        )
                                        nc.vector.tensor_mul(
                                            tgt, u_sb[:, n * 512 : (n + 1) * 512], bc[:]
                                        )
                                    if hh:
                                        nc.gpsimd.dma_start(att_sb[64:128, t, :], stage[:])

                                # v projection (overlaps phase 0 on PE; av slots)
                                for m in range(NT):
                                    ps = aps.tile([128, 512], F32, tag="av", bufs=4, name=f"pv{m}")
                                    for j in range(8):
                                        nc.tensor.matmul(
                                            ps[:, 0:DSL],
                                            vt_sb[:, j, m * 128 : (m + 1) * 128],
                                            wv_sb[:, j, :],
                                            start=(j == 0),
                                            stop=(j == 7),
                                        )
                                    nc.vector.tensor_copy(
                                        vh_sb[:, m, :].rearrange("p (h x) -> p h x", h=4)[
                                            :, :, 0:64
                                        ],
                                        ps[:, 0:DSL].rearrange("p (h x) -> p h x", h=4),
                                    )

                                for h in range(4):
                                    if h > 0:
                                        avs[h - 1] = [
                                            aps.tile([128, 512], F32, tag="av", bufs=4,
                                                     name=f"av{h - 1}{n}")
                                            for n in range(NC4)
                                        ]
                                    for m in range(NT):
                                        if h > 0:
                                            av_step(h - 1, m)
                                        s_step(h, m)
                                    if h > 0:
                                        ucopy(h - 1)
                                    if h > 1:
                                        normrest(h - 2)
                                avs[3] = [
                                    aps.tile([128, 512], F32, tag="av", bufs=4, name=f"av3{n}")
                                    for n in range(NC4)
                                ]
                                for m in range(NT):
                                    av_step(3, m)
                                ucopy(3)
                                normrest(2)
                                normrest(3)

                                # ---------------- output projection ----------------
                                for s in range(NT):
                                    ot = ap.tile([128, 1024], F32, tag="o", bufs=3, name=f"ot{s}")
                                    for c in range(2):
                                        op = aps.tile([128, 512], F32, tag="av", bufs=4, name=f"op{s}{c}")
                                        for kt2 in range(2):
                                            nc.tensor.matmul(
                                                op[:],
                                                att_sb[:, kt2, s * 128 : (s + 1) * 128],
                                                wo_sb[:, kt2, c * 512 : (c + 1) * 512],
                                                start=(kt2 == 0),
                                                stop=(kt2 == 1),
                                            )
                                        if c == 0:
                                            nc.vector.tensor_copy(ot[:, 0:512], op[:])
                                        else:
                                            nc.scalar.copy(ot[:, 512:1024], op[:])
                                    eng = nc.sync if s % 2 == 0 else nc.gpsimd
                                    eng.dma_start(out_d[s * 128 : (s + 1) * 128, :], ot[:])
    nc.compile()
    names = dict(
        qT=qT_d.name, kT=kT_d.name, vT=vT_d.name,
        wq=wq_d.name, wk=wk_d.name, wv=wv_d.name, wo=wo_d.name, out=out_d.name,
    )
    return nc, names


def _dev_layout_x(x):
    # [seq, dm] f32 -> transposed [dm, seq] -> [128, 8, seq] bf16
    xt = np.ascontiguousarray(x.T).astype(BF16)
    return np.ascontiguousarray(xt.reshape(8, 128, SEQ).swapaxes(0, 1))


def _dev_layout_w(w):
    # [256, dm] slice -> W.T [dm, 256] -> [128, 8, 256] bf16
    wt = np.ascontiguousarray(w.T).astype(BF16)
    return np.ascontiguousarray(wt.reshape(8, 128, DSL).swapaxes(0, 1))


def kernel(q, k, v, W_q, b_q, W_k, b_k, W_v, b_v, W_o, b_o, trace=False):
    if "nc" not in _cache:
        _cache["nc"], _cache["names"] = _build()
    nc, names = _cache["nc"], _cache["names"]

    q, k, v = np.asarray(q), np.asarray(k), np.asarray(v)
    in_maps = []
    for core in range(8):
        b, g = core // 4, core % 4
        s256 = slice(256 * g, 256 * (g + 1))
        wo_slice = np.ascontiguousarray(np.asarray(W_o)[:, s256].T).astype(BF16)
        in_maps.append({
            names["qT"]: _dev_layout_x(q[b]),
            names["kT"]: _dev_layout_x(k[b]),
            names["vT"]: _dev_layout_x(v[b]),
            names["wq"]: _dev_layout_w(np.asarray(W_q)[s256] * 0.125),
            names["wk"]: _dev_layout_w(np.asarray(W_k)[s256]),
            names["wv"]: _dev_layout_w(np.asarray(W_v)[s256]),
            names["wo"]: np.ascontiguousarray(
                wo_slice.reshape(2, 128, DM).swapaxes(0, 1)
            ),
        })

    res = run_bass_kernel_spmd(nc, in_maps, core_ids=list(range(8)), trace=trace)
    out = np.zeros((2, SEQ, DM), np.float32)
    for core in range(8):
        out[core // 4] += res.results[core][names["out"]]
    out += np.asarray(b_o)[None, None, :].astype(np.float32)
    _cache["last_res"] = res
    return out



# revision 5
# speedup vs baseline: 1.3982x; 1.3982x over previous
"""Multi-head attention (bs=2, seq=2048, d_model=1024, 16 heads) on 8 NeuronCores.

Sharding: core = b*4 + g  (b = batch 0..1, g = head-group 0..3, 4 heads each).
Per core, for batch b and head slice s256 = [256g, 256g+256):
  qhT [256, 2048] = (0.125*W_q[s256]) @ q[b].T      (scores scale folded into W_q)
  khT [256, 2048] = W_k[s256] @ k[b].T
  vh  [2048, 260] = v[b] @ W_v[s256].T              (+ ones column per head)
  per head: S^T = khT-slice.T @ qhT -> exp -> P^T (bf16)
            attnU^T[65, sq] = vh_aug.T @ P^T        (row 64 = softmax sums)
            normalize: gpsimd partition_broadcast of sums + DVE recip/mul
  out_partial [2048, 1024] = attnN @ W_o[:, s256].T   (f32)
Host sums the 4 partials per batch and adds b_o.

Pipeline: inputs DMA'd in 512-col slices so projections start early; the
exp stream on the Scalar engine (the throughput floor, ~1us per
[128,1024] tile) runs continuously; per-head AV accumulation streams
trail the S/exp stream one head at a time (PSUM: 4 banks S + 4 banks AV);
the last head's AV runs n-inner so the output projection and store
pipeline with it.
"""

import sys

sys.path.insert(0, "/opt/trn_rl_repo")

import numpy as np
import ml_dtypes

import concourse.bass as bass
import concourse.mybir as mybir
import concourse.tile as tile
from concourse import bacc
from concourse.bass_utils import run_bass_kernel_spmd

BF16 = ml_dtypes.bfloat16
F32 = mybir.dt.float32
BF = mybir.dt.bfloat16

SEQ = 2048
DM = 1024
DSL = 256            # head dims per core
NT = SEQ // 128      # 16 seq tiles
NC4 = 4              # seq chunks of 512

_cache = {}


def _build(reps=1):
    nc = bacc.Bacc(None, target_bir_lowering=False, debug=False)
    with tile.TileContext(nc) as tc:
        with tc.tile_pool(name="dram", bufs=1, space="DRAM") as dram:
            qT_d = dram.tile([128, 8, SEQ], BF, kind="ExternalInput", tag="qT")
            kT_d = dram.tile([128, 8, SEQ], BF, kind="ExternalInput", tag="kT")
            vT_d = dram.tile([128, 8, SEQ], BF, kind="ExternalInput", tag="vT")
            wq_d = dram.tile([128, 8, DSL], BF, kind="ExternalInput", tag="wq")
            wk_d = dram.tile([128, 8, DSL], BF, kind="ExternalInput", tag="wk")
            wv_d = dram.tile([128, 8, DSL], BF, kind="ExternalInput", tag="wv")
            wo_d = dram.tile([128, 2, DM], BF, kind="ExternalInput", tag="wo")
            out_d = dram.tile([SEQ, DM], F32, kind="ExternalOutput", tag="out")

            with tc.tile_pool(name="const", bufs=1) as cp:
                wo_sb = cp.tile([128, 2, DM], BF, tag="cwo")
                nc.gpsimd.dma_start(wo_sb[:], wo_d[:])

                with tc.tile_pool(name="persist", bufs=1) as pp:
                    qh_sb = pp.tile([128, 2, SEQ], BF, tag="qh")
                    kh_sb = pp.tile([128, 2, SEQ], BF, tag="kh")
                    vh_sb = pp.tile([128, NT, 260], BF, tag="vh")
                    att_sb = pp.tile([128, 2, SEQ], BF, tag="att")
                    vh_ones = vh_sb[:].rearrange(
                        "p m (h x) -> p m h x", h=4
                    )[:, :, :, 64:65]
                    nc.vector.memset(vh_ones, 1.0)

                    for _rep in range(reps):
                        with (
                            tc.tile_pool(name="aps", bufs=1, space="PSUM") as aps,
                            tc.tile_pool(name="wkp", bufs=1) as wk,
                        ):
                            pts = {}
                            avs = {}

                            def s_step(hh, c, m):
                                # scores for head hh, q-columns [1024c, 1024c+1024)
                                t, p0 = hh // 2, 64 * (hh % 2)
                                pt = wk.tile([128, 1024], BF, tag="pt", bufs=13,
                                             name=f"pt{hh}_{c}_{m}")
                                pts[(hh, c, m)] = pt
                                sp = aps.tile([128, 1024], F32, tag="sp",
                                              bufs=2, name=f"sp{hh}{c}{m}")
                                for nn in range(2):
                                    qsl = slice(c * 1024 + nn * 512,
                                                c * 1024 + (nn + 1) * 512)
                                    nc.tensor.matmul(
                                        sp[:, nn * 512:(nn + 1) * 512],
                                        kh_sb[p0:p0 + 64, t, m * 128:(m + 1) * 128],
                                        qh_sb[p0:p0 + 64, t, qsl],
                                        start=True, stop=True,
                                    )
                                nc.scalar.activation(
                                    pt[:], sp[:],
                                    mybir.ActivationFunctionType.Exp,
                                )

                            def av_alloc(hh, c):
                                for n in (2 * c, 2 * c + 1):
                                    avs[(hh, n)] = aps.tile(
                                        [128, 512], F32, tag="av", bufs=4,
                                        name=f"av{hh}_{n}")

                            def av_step(hh, c, m):
                                # accumulate attnU for n-chunks {2c, 2c+1}
                                pt = pts[(hh, c, m)]
                                for nn in range(2):
                                    nc.tensor.matmul(
                                        avs[(hh, 2 * c + nn)][0:65, :],
                                        vh_sb[:, m, 65 * hh:65 * hh + 65],
                                        pt[:, nn * 512:(nn + 1) * 512],
                                        start=(m == 0), stop=(m == NT - 1),
                                    )

                            def norm_chunk(hh, n):
                                t, odd = hh // 2, hh % 2
                                sl = slice(n * 512, (n + 1) * 512)
                                u = wk.tile([65, 512], F32, tag="u", bufs=3,
                                            name=f"u{hh}_{n}")
                                nc.vector.tensor_copy(u[:], avs[(hh, n)][0:65, :])
                                s0 = wk.tile([1, 512], F32, tag="s0", bufs=2,
                                             name=f"s0{hh}_{n}")
                                nc.vector.tensor_copy(s0[:], avs[(hh, n)][64:65, :])
                                sbc = wk.tile([64, 512], F32, tag="sbc", bufs=2,
                                              name=f"sbc{hh}_{n}")
                                nc.gpsimd.partition_broadcast(
                                    sbc[:], s0[:], channels=64
                                )
                                rbc = wk.tile([64, 512], F32, tag="rbc", bufs=2,
                                              name=f"rbc{hh}_{n}")
                                nc.vector.reciprocal(rbc[:], sbc[:])
                                if odd:
                                    stg = wk.tile([64, 512], BF, tag="stg",
                                                  bufs=2, name=f"stg{hh}_{n}")
                                    nc.vector.tensor_mul(stg[:], u[0:64, :], rbc[:])
                                    nc.sync.dma_start(att_sb[64:128, t, sl], stg[:])
                                else:
                                    nc.vector.tensor_mul(
                                        att_sb[0:64, t, sl], u[0:64, :], rbc[:]
                                    )

                            out_engs = [nc.sync, nc.gpsimd, nc.scalar]

                            def o_step(s):
                                op = aps.tile([128, 1024], F32, tag="sp", bufs=2,
                                              name=f"op{s}")
                                for c in range(2):
                                    for kt2 in range(2):
                                        nc.tensor.matmul(
                                            op[:, c * 512:(c + 1) * 512],
                                            att_sb[:, kt2, s * 128:(s + 1) * 128],
                                            wo_sb[:, kt2, c * 512:(c + 1) * 512],
                                            start=(kt2 == 0), stop=(kt2 == 1),
                                        )
                                ot = wk.tile([128, 1024], F32, tag="ot", bufs=3,
                                             name=f"ot{s}")
                                nc.vector.tensor_copy(ot[:], op[:])
                                out_engs[s % 3].dma_start(
                                    out_d[s * 128:(s + 1) * 128, :], ot[:]
                                )

                            # ---------------- input DMA + q/k projections ----
                            with tc.tile_pool(name="io", bufs=1) as io:
                                wq_sb = io.tile([128, 8, DSL], BF, tag="cwq")
                                wk_sb = io.tile([128, 8, DSL], BF, tag="cwk")
                                nc.scalar.dma_start(wq_sb[:], wq_d[:])
                                nc.scalar.dma_start(wk_sb[:], wk_d[:])
                                qt_sb = io.tile([128, 8, SEQ], BF, tag="qt")
                                kt_sb = io.tile([128, 8, SEQ], BF, tag="kt")
                                vt_sb = io.tile([128, 8, SEQ], BF, tag="vt")
                                wv_sb = io.tile([128, 8, DSL], BF, tag="cwv")
                                nc.gpsimd.dma_start(wv_sb[:], wv_d[:])
                                # one queue, priority order: q/k slices early,
                                # v interleaved behind
                                order = [("q", 0), ("k", 0), ("q", 1), ("k", 1),
                                         ("v", 0), ("q", 2), ("k", 2), ("v", 1),
                                         ("q", 3), ("k", 3), ("v", 2), ("v", 3)]
                                srcs = {"q": (qt_sb, qT_d), "k": (kt_sb, kT_d),
                                        "v": (vt_sb, vT_d)}
                                for wch, n in order:
                                    sb, dr = srcs[wch]
                                    sl = slice(n * 512, (n + 1) * 512)
                                    nc.sync.dma_start(sb[:, :, sl], dr[:, :, sl])

                                def v_step(m):
                                    pv = aps.tile([128, 512], F32, tag="av",
                                                  bufs=4, name=f"pv{m}")
                                    for j in range(8):
                                        nc.tensor.matmul(
                                            pv[:, 0:DSL],
                                            vt_sb[:, j, m * 128:(m + 1) * 128],
                                            wv_sb[:, j, :],
                                            start=(j == 0), stop=(j == 7),
                                        )
                                    nc.vector.tensor_copy(
                                        vh_sb[:, m, :].rearrange(
                                            "p (h x) -> p h x", h=4
                                        )[:, :, 0:64],
                                        pv[:, 0:DSL].rearrange(
                                            "p (h x) -> p h x", h=4
                                        ),
                                    )

                                def projqk(n):
                                    sl = slice(n * 512, (n + 1) * 512)
                                    pq = [aps.tile([128, 512], F32, tag="av",
                                                   bufs=4, name=f"pq{n}{m}")
                                          for m in range(2)]
                                    pk = [aps.tile([128, 512], F32, tag="av",
                                                   bufs=4, name=f"pk{n}{m}")
                                          for m in range(2)]
                                    for j in range(8):
                                        for m in range(2):
                                            nc.tensor.matmul(
                                                pq[m][:],
                                                wq_sb[:, j, m * 128:(m + 1) * 128],
                                                qt_sb[:, j, sl],
                                                start=(j == 0), stop=(j == 7),
                                            )
                                        for m in range(2):
                                            nc.tensor.matmul(
                                                pk[m][:],
                                                wk_sb[:, j, m * 128:(m + 1) * 128],
                                                kt_sb[:, j, sl],
                                                start=(j == 0), stop=(j == 7),
                                            )
                                    for m in range(2):
                                        nc.vector.tensor_copy(
                                            qh_sb[:, m, sl], pq[m][:]
                                        )
                                        nc.vector.tensor_copy(
                                            kh_sb[:, m, sl], pk[m][:]
                                        )

                                projqk(0)
                                projqk(1)

                                # ---- h0 pass 0 (c=0) with v-proj + late projqk
                                PROJ_AT = {5: 2, 10: 3}
                                V_AT = {3: [0, 1], 4: [2, 3], 8: [4, 5],
                                        9: [6, 7], 12: [8, 9], 13: [10, 11],
                                        14: [12, 13], 15: [14, 15]}
                                AV0_P0 = {12: [0], 13: [1, 2], 14: [3, 4],
                                          15: [5, 6]}
                                for m in range(NT):
                                    s_step(0, 0, m)
                                    if m in PROJ_AT:
                                        projqk(PROJ_AT[m])
                                    for vm in V_AT.get(m, []):
                                        v_step(vm)
                                    if m == 12:
                                        av_alloc(0, 0)
                                    for k in AV0_P0.get(m, []):
                                        av_step(0, 0, k)

                                # ---- h0 pass 1 (c=1)
                                AV0_P1 = {0: [7, 8], 1: [9, 10], 2: [11, 12],
                                          3: [13, 14], 4: [15]}
                                for m in range(NT):
                                    s_step(0, 1, m)
                                    for k in AV0_P1.get(m, []):
                                        av_step(0, 0, k)
                                    if m == 1:
                                        av_alloc(0, 1)
                                    if m >= 2:
                                        av_step(0, 1, m - 2)
                                    if m == 5:
                                        norm_chunk(0, 0)
                                    if m == 6:
                                        norm_chunk(0, 1)

                            # ---- heads 1..3: steady two-pass pipeline ----
                            for hh in range(1, 4):
                                ph = hh - 1
                                for m in range(NT):  # pass 0
                                    s_step(hh, 0, m)
                                    if m == 0:
                                        av_step(ph, 1, 14)
                                        av_step(ph, 1, 15)
                                    if m == 1:
                                        norm_chunk(ph, 2)
                                    if m == 2:
                                        norm_chunk(ph, 3)
                                        av_alloc(hh, 0)
                                    if m >= 3:
                                        av_step(hh, 0, m - 3)
                                for m in range(NT):  # pass 1
                                    s_step(hh, 1, m)
                                    if m == 0:
                                        av_step(hh, 0, 13)
                                        av_step(hh, 0, 14)
                                    if m == 1:
                                        av_step(hh, 0, 15)
                                        av_alloc(hh, 1)
                                    if m >= 2:
                                        av_step(hh, 1, m - 2)
                                    if m == 3:
                                        norm_chunk(hh, 0)
                                    if m == 4:
                                        norm_chunk(hh, 1)

                            # ---- tail: finish h3, output projection ----
                            av_step(3, 1, 14)
                            av_step(3, 1, 15)
                            for s in range(8):
                                o_step(s)
                            norm_chunk(3, 2)
                            norm_chunk(3, 3)
                            for s in range(8, 16):
                                o_step(s)
    nc.compile()
    names = dict(
        qT=qT_d.name, kT=kT_d.name, vT=vT_d.name,
        wq=wq_d.name, wk=wk_d.name, wv=wv_d.name, wo=wo_d.name, out=out_d.name,
    )
    return nc, names


def _dev_layout_x(x):
    # [seq, dm] f32 -> transposed [dm, seq] -> [128, 8, seq] bf16
    xt = np.ascontiguousarray(x.T).astype(BF16)
    return np.ascontiguousarray(xt.reshape(8, 128, SEQ).swapaxes(0, 1))


def _dev_layout_w(w):
    # [256, dm] slice -> W.T [dm, 256] -> [128, 8, 256] bf16
    wt = np.ascontiguousarray(w.T).astype(BF16)
    return np.ascontiguousarray(wt.reshape(8, 128, DSL).swapaxes(0, 1))


def kernel(q, k, v, W_q, b_q, W_k, b_k, W_v, b_v, W_o, b_o, trace=False):
    if "nc" not in _cache:
        _cache["nc"], _cache["names"] = _build()
    nc, names = _cache["nc"], _cache["names"]

    q, k, v = np.asarray(q), np.asarray(k), np.asarray(v)
    in_maps = []
    for core in range(8):
        b, g = core // 4, core % 4
        s256 = slice(256 * g, 256 * (g + 1))
        wo_slice = np.ascontiguousarray(np.asarray(W_o)[:, s256].T).astype(BF16)
        in_maps.append({
            names["qT"]: _dev_layout_x(q[b]),
            names["kT"]: _dev_layout_x(k[b]),
            names["vT"]: _dev_layout_x(v[b]),
            names["wq"]: _dev_layout_w(np.asarray(W_q)[s256] * 0.125),
            names["wk"]: _dev_layout_w(np.asarray(W_k)[s256]),
            names["wv"]: _dev_layout_w(np.asarray(W_v)[s256]),
            names["wo"]: np.ascontiguousarray(
                wo_slice.reshape(2, 128, DM).swapaxes(0, 1)
            ),
        })

    res = run_bass_kernel_spmd(nc, in_maps, core_ids=list(range(8)), trace=trace)
    out = np.zeros((2, SEQ, DM), np.float32)
    for core in range(8):
        out[core // 4] += res.results[core][names["out"]]
    out += np.asarray(b_o)[None, None, :].astype(np.float32)
    _cache["last_res"] = res
    return out


# revision 10
# speedup vs baseline: 1.4344x; 1.0258x over previous
"""Multi-head attention (bs=2, seq=2048, d_model=1024, 16 heads) on 8 NeuronCores.

Sharding: core = b*4 + g  (b = batch 0..1, g = head-group 0..3, 4 heads each).
Per core, for batch b and head slice s256 = [256g, 256g+256):
  qhT [256, 2048] = (0.125*W_q[s256]) @ q[b].T      (scores scale folded into W_q)
  khT [256, 2048] = W_k[s256] @ k[b].T
  vh  [2048, 260] = v[b] @ W_v[s256].T              (+ ones column per head)
  per head: S^T = khT-slice.T @ qhT -> exp -> P^T (bf16)
            attnU^T[65, sq] = vh_aug.T @ P^T        (row 64 = softmax sums)
            normalize: gpsimd partition_broadcast of sums + DVE recip/mul
  out_partial [2048, 1024] = attnN @ W_o[:, s256].T   (f32)
Host sums the 4 partials per batch and adds b_o.

Pipeline: inputs DMA'd in 512-col slices so projections start early; the
exp stream on the Scalar engine (the throughput floor, ~1us per
[128,1024] tile) runs continuously; per-head AV accumulation streams
trail the S/exp stream one head at a time (PSUM: 4 banks S + 4 banks AV);
the last head's AV runs n-inner so the output projection and store
pipeline with it.
"""

import sys

sys.path.insert(0, "/opt/trn_rl_repo")

import numpy as np
import ml_dtypes

import concourse.bass as bass
import concourse.mybir as mybir
import concourse.tile as tile
from concourse import bacc
from concourse.bass_utils import run_bass_kernel_spmd

BF16 = ml_dtypes.bfloat16
F32 = mybir.dt.float32
BF = mybir.dt.bfloat16

SEQ = 2048
DM = 1024
DSL = 256            # head dims per core
NT = SEQ // 128      # 16 seq tiles
NC4 = 4              # seq chunks of 512

_cache = {}


def _build(reps=1):
    nc = bacc.Bacc(None, target_bir_lowering=False, debug=False)
    with tile.TileContext(nc) as tc:
        with tc.tile_pool(name="dram", bufs=1, space="DRAM") as dram:
            qT_d = dram.tile([128, 8, SEQ], BF, kind="ExternalInput", tag="qT")
            kT_d = dram.tile([128, 8, SEQ], BF, kind="ExternalInput", tag="kT")
            vT_d = dram.tile([128, 8, SEQ], BF, kind="ExternalInput", tag="vT")
            wq_d = dram.tile([128, 8, DSL], BF, kind="ExternalInput", tag="wq")
            wk_d = dram.tile([128, 8, DSL], BF, kind="ExternalInput", tag="wk")
            wv_d = dram.tile([128, 8, DSL], BF, kind="ExternalInput", tag="wv")
            wo_d = dram.tile([128, 2, DM], BF, kind="ExternalInput", tag="wo")
            out_d = dram.tile([SEQ, DM], F32, kind="ExternalOutput", tag="out")

            with tc.tile_pool(name="const", bufs=1) as cp:
                wo_sb = cp.tile([128, 2, DM], BF, tag="cwo")
                nc.gpsimd.dma_start(wo_sb[:], wo_d[:])

                with tc.tile_pool(name="persist", bufs=1) as pp:
                    qh_sb = pp.tile([128, 2, SEQ], BF, tag="qh")
                    kh_sb = pp.tile([128, 2, SEQ], BF, tag="kh")
                    vh_sb = pp.tile([128, NT, 260], BF, tag="vh")
                    att_sb = pp.tile([128, 2, SEQ], BF, tag="att")
                    vh_ones = vh_sb[:].rearrange(
                        "p m (h x) -> p m h x", h=4
                    )[:, :, :, 64:65]
                    nc.vector.memset(vh_ones, 1.0)

                    for _rep in range(reps):
                        with (
                            tc.tile_pool(name="aps", bufs=1, space="PSUM") as aps,
                            tc.tile_pool(name="wkp", bufs=1) as wk,
                        ):
                            pts = {}
                            avs = {}

                            def s_step(hh, c, m):
                                # scores for head hh, q-columns [1024c, 1024c+1024)
                                t, p0 = hh // 2, 64 * (hh % 2)
                                pt = wk.tile([128, 1024], BF, tag="pt", bufs=13,
                                             name=f"pt{hh}_{c}_{m}")
                                pts[(hh, c, m)] = pt
                                sp = aps.tile([128, 1024], F32, tag="sp",
                                              bufs=2, name=f"sp{hh}{c}{m}")
                                for nn in range(2):
                                    qsl = slice(c * 1024 + nn * 512,
                                                c * 1024 + (nn + 1) * 512)
                                    nc.tensor.matmul(
                                        sp[:, nn * 512:(nn + 1) * 512],
                                        kh_sb[p0:p0 + 64, t, m * 128:(m + 1) * 128],
                                        qh_sb[p0:p0 + 64, t, qsl],
                                        start=True, stop=True,
                                    )
                                nc.scalar.activation(
                                    pt[:], sp[:],
                                    mybir.ActivationFunctionType.Exp,
                                )

                            def av_alloc(hh, c):
                                for n in (2 * c, 2 * c + 1):
                                    avs[(hh, n)] = aps.tile(
                                        [128, 512], F32, tag="av", bufs=4,
                                        name=f"av{hh}_{n}")

                            def av_step(hh, c, m):
                                # accumulate attnU for n-chunks {2c, 2c+1}
                                pt = pts[(hh, c, m)]
                                for nn in range(2):
                                    nc.tensor.matmul(
                                        avs[(hh, 2 * c + nn)][0:65, :],
                                        vh_sb[:, m, 65 * hh:65 * hh + 65],
                                        pt[:, nn * 512:(nn + 1) * 512],
                                        start=(m == 0), stop=(m == NT - 1),
                                    )

                            def norm_chunk(hh, n):
                                t, odd = hh // 2, hh % 2
                                sl = slice(n * 512, (n + 1) * 512)
                                u = wk.tile([65, 512], F32, tag="u", bufs=3,
                                            name=f"u{hh}_{n}")
                                nc.vector.tensor_copy(u[:], avs[(hh, n)][0:65, :])
                                s0 = wk.tile([1, 512], F32, tag="s0", bufs=2,
                                             name=f"s0{hh}_{n}")
                                nc.vector.tensor_copy(s0[:], avs[(hh, n)][64:65, :])
                                sbc = wk.tile([64, 512], F32, tag="sbc", bufs=2,
                                              name=f"sbc{hh}_{n}")
                                nc.gpsimd.partition_broadcast(
                                    sbc[:], s0[:], channels=64
                                )
                                rbc = wk.tile([64, 512], F32, tag="rbc", bufs=2,
                                              name=f"rbc{hh}_{n}")
                                nc.vector.reciprocal_approx_fast(rbc[:], sbc[:])
                                if odd:
                                    stg = wk.tile([64, 512], BF, tag="stg",
                                                  bufs=2, name=f"stg{hh}_{n}")
                                    nc.vector.tensor_mul(stg[:], u[0:64, :], rbc[:])
                                    nc.sync.dma_start(att_sb[64:128, t, sl], stg[:])
                                else:
                                    nc.vector.tensor_mul(
                                        att_sb[0:64, t, sl], u[0:64, :], rbc[:]
                                    )

                            out_engs = [nc.sync, nc.gpsimd, nc.scalar]

                            def o_step(s):
                                ot = wk.tile([128, 1024], F32, tag="ot", bufs=3,
                                             name=f"ot{s}")
                                for c in range(2):
                                    op = aps.tile([128, 512], F32, tag="av",
                                                  bufs=4, name=f"op{s}_{c}")
                                    for kt2 in range(2):
                                        nc.tensor.matmul(
                                            op[:],
                                            att_sb[:, kt2, s * 128:(s + 1) * 128],
                                            wo_sb[:, kt2, c * 512:(c + 1) * 512],
                                            start=(kt2 == 0), stop=(kt2 == 1),
                                        )
                                    nc.vector.tensor_copy(
                                        ot[:, c * 512:(c + 1) * 512], op[:]
                                    )
                                out_engs[s % 3].dma_start(
                                    out_d[s * 128:(s + 1) * 128, :], ot[:]
                                )

                            # ---------------- input DMA + q/k projections ----
                            with tc.tile_pool(name="io", bufs=1) as io:
                                wq_sb = io.tile([128, 8, DSL], BF, tag="cwq")
                                wk_sb = io.tile([128, 8, DSL], BF, tag="cwk")
                                nc.scalar.dma_start(wq_sb[:], wq_d[:])
                                nc.scalar.dma_start(wk_sb[:], wk_d[:])
                                qt_sb = io.tile([128, 8, SEQ], BF, tag="qt")
                                kt_sb = io.tile([128, 8, SEQ], BF, tag="kt")
                                vt_sb = io.tile([128, 8, SEQ], BF, tag="vt")
                                wv_sb = io.tile([128, 8, DSL], BF, tag="cwv")
                                nc.gpsimd.dma_start(wv_sb[:], wv_d[:])
                                # one queue (scalar: cheap issues), priority
                                # order: q/k slices early, v interleaved behind
                                order = [("q", 0), ("k", 0), ("q", 1), ("k", 1),
                                         ("v", 0), ("q", 2), ("k", 2), ("v", 1),
                                         ("q", 3), ("k", 3), ("v", 2), ("v", 3)]
                                srcs = {"q": (qt_sb, qT_d), "k": (kt_sb, kT_d),
                                        "v": (vt_sb, vT_d)}
                                for wch, n in order:
                                    sb, dr = srcs[wch]
                                    sl = slice(n * 512, (n + 1) * 512)
                                    nc.scalar.dma_start(sb[:, :, sl], dr[:, :, sl])

                                def v_step(m):
                                    pv = aps.tile([128, 512], F32, tag="av",
                                                  bufs=4, name=f"pv{m}")
                                    for j in range(8):
                                        nc.tensor.matmul(
                                            pv[:, 0:DSL],
                                            vt_sb[:, j, m * 128:(m + 1) * 128],
                                            wv_sb[:, j, :],
                                            start=(j == 0), stop=(j == 7),
                                        )
                                    nc.vector.tensor_copy(
                                        vh_sb[:, m, :].rearrange(
                                            "p (h x) -> p h x", h=4
                                        )[:, :, 0:64],
                                        pv[:, 0:DSL].rearrange(
                                            "p (h x) -> p h x", h=4
                                        ),
                                    )

                                def proj1(n, w_sb, x_sb, o_sb, pfx):
                                    sl = slice(n * 512, (n + 1) * 512)
                                    ps = [aps.tile([128, 512], F32, tag="av",
                                                   bufs=4, name=f"{pfx}{n}{m}")
                                          for m in range(2)]
                                    for j in range(8):
                                        for m in range(2):
                                            nc.tensor.matmul(
                                                ps[m][:],
                                                w_sb[:, j, m * 128:(m + 1) * 128],
                                                x_sb[:, j, sl],
                                                start=(j == 0), stop=(j == 7),
                                            )
                                    for m in range(2):
                                        nc.vector.tensor_copy(
                                            o_sb[:, m, sl], ps[m][:]
                                        )

                                def projq(n):
                                    proj1(n, wq_sb, qt_sb, qh_sb, "pq")

                                def projk(n):
                                    proj1(n, wk_sb, kt_sb, kh_sb, "pk")

                                projq(0)
                                projk(0)
                                projq(1)
                                projk(1)

                                # ---- h0 pass 0 (c=0) with v-proj + late projqk
                                PROJ_AT = {3: lambda: projq(2),
                                           5: lambda: projk(2),
                                           7: lambda: projq(3),
                                           9: lambda: projk(3)}
                                V_AT = {2: [0], 3: [1], 4: [2], 5: [3], 6: [4],
                                        7: [5], 8: [6], 9: [7], 10: [8, 9],
                                        11: [10, 11], 12: [12, 13], 13: [14, 15]}
                                AV0_P0 = {13: [0], 14: [1, 2], 15: [3, 4]}
                                for m in range(NT):
                                    s_step(0, 0, m)
                                    if m in PROJ_AT:
                                        PROJ_AT[m]()
                                    for vm in V_AT.get(m, []):
                                        v_step(vm)
                                    if m == 13:
                                        av_alloc(0, 0)
                                    for k in AV0_P0.get(m, []):
                                        av_step(0, 0, k)

                                # ---- h0 pass 1 (c=1)
                                AV0_P1 = {0: [5, 6], 1: [7, 8], 2: [9, 10],
                                          3: [11, 12], 4: [13, 14], 5: [15]}
                                for m in range(NT):
                                    s_step(0, 1, m)
                                    for k in AV0_P1.get(m, []):
                                        av_step(0, 0, k)
                                    if m == 1:
                                        av_alloc(0, 1)
                                    if m >= 2:
                                        av_step(0, 1, m - 2)
                                    if m == 6:
                                        norm_chunk(0, 0)
                                    if m == 7:
                                        norm_chunk(0, 1)

                            # ---- heads 1..3: steady two-pass pipeline ----
                            for hh in range(1, 4):
                                ph = hh - 1
                                for m in range(NT):  # pass 0
                                    s_step(hh, 0, m)
                                    if m == 0:
                                        av_step(ph, 1, 14)
                                        av_step(ph, 1, 15)
                                    if m == 1:
                                        norm_chunk(ph, 2)
                                    if m == 2:
                                        norm_chunk(ph, 3)
                                        av_alloc(hh, 0)
                                    if m >= 3:
                                        av_step(hh, 0, m - 3)
                                for m in range(NT):  # pass 1
                                    s_step(hh, 1, m)
                                    if m == 0:
                                        av_step(hh, 0, 13)
                                        av_step(hh, 0, 14)
                                    if m == 1:
                                        av_step(hh, 0, 15)
                                        av_alloc(hh, 1)
                                    if m >= 2:
                                        av_step(hh, 1, m - 2)
                                    if m == 3:
                                        norm_chunk(hh, 0)
                                    if m == 4:
                                        norm_chunk(hh, 1)
                                    if hh == 3 and 6 <= m <= 13:
                                        o_step(m - 6)

                            # ---- tail: finish h3, output projection ----
                            av_step(3, 1, 14)
                            av_step(3, 1, 15)
                            norm_chunk(3, 2)
                            norm_chunk(3, 3)
                            for s in range(8, 16):
                                o_step(s)
    nc.compile()
    names = dict(
        qT=qT_d.name, kT=kT_d.name, vT=vT_d.name,
        wq=wq_d.name, wk=wk_d.name, wv=wv_d.name, wo=wo_d.name, out=out_d.name,
    )
    return nc, names


def _dev_layout_x(x):
    # [seq, dm] f32 -> transposed [dm, seq] -> [128, 8, seq] bf16
    xt = np.ascontiguousarray(x.T).astype(BF16)
    return np.ascontiguousarray(xt.reshape(8, 128, SEQ).swapaxes(0, 1))


def _dev_layout_w(w):
    # [256, dm] slice -> W.T [dm, 256] -> [128, 8, 256] bf16
    wt = np.ascontiguousarray(w.T).astype(BF16)
    return np.ascontiguousarray(wt.reshape(8, 128, DSL).swapaxes(0, 1))


def kernel(q, k, v, W_q, b_q, W_k, b_k, W_v, b_v, W_o, b_o, trace=False):
    if "nc" not in _cache:
        _cache["nc"], _cache["names"] = _build()
    nc, names = _cache["nc"], _cache["names"]

    q, k, v = np.asarray(q), np.asarray(k), np.asarray(v)
    in_maps = []
    for core in range(8):
        b, g = core // 4, core % 4
        s256 = slice(256 * g, 256 * (g + 1))
        wo_slice = np.ascontiguousarray(np.asarray(W_o)[:, s256].T).astype(BF16)
        in_maps.append({
            names["qT"]: _dev_layout_x(q[b]),
            names["kT"]: _dev_layout_x(k[b]),
            names["vT"]: _dev_layout_x(v[b]),
            names["wq"]: _dev_layout_w(np.asarray(W_q)[s256] * 0.125),
            names["wk"]: _dev_layout_w(np.asarray(W_k)[s256]),
            names["wv"]: _dev_layout_w(np.asarray(W_v)[s256]),
            names["wo"]: np.ascontiguousarray(
                wo_slice.reshape(2, 128, DM).swapaxes(0, 1)
            ),
        })

    res = run_bass_kernel_spmd(nc, in_maps, core_ids=list(range(8)), trace=trace)
    out = np.zeros((2, SEQ, DM), np.float32)
    for core in range(8):
        out[core // 4] += res.results[core][names["out"]]
    out += np.asarray(b_o)[None, None, :].astype(np.float32)
    _cache["last_res"] = res
    return out
